# revision 39
# baseline (speedup 1.0000x reference)
"""Locally-connected network (28x28 -> lc3x3 -> lc3x3 -> fc10) on 8 TRN2 cores.

The reference network is linear (two locally-connected layers + FC, no
activations), so the host folds it into one affine map
    out[b, :] = x[b, :784] @ M + c          (M: [784, 10], c: [10])
in float64. The device kernel is pure data-parallel over each core's
1024-sample shard and is stream-bound: x bytes dominate, so precision is
allocated by row energy ||M[k]||^2:

  * top 224 pixel rows stream fp16 (x and M fp16),
  * remaining 560 rows (+16 pad rows) stream fp8e4m3; their M rows are
    applied twice, as fp8 high part M_h plus fp8 residual M_r = M - M_h, so
    the weight-quantization error cancels and only x's fp8 error remains.
  * the bias needs no extra pass: the first pad row streams constant 1.0
    with c as its (h+r) weight row.

Measured end-to-end relative error ~1.5e-2 (gate 2e-2).

Matmul orientation: the x block is the STATIONARY operand [K pixels, 128
samples] and the M tile is the MOVING operand [K, 10], so PSUM holds
[128 samples, 10] per block and each matmul streams only 10 columns. Per
128-sample block: 19 matmuls; two blocks share one PSUM bank so one DVE
copy drains a pair (amortizing the PSUM access bubble).

Input layout (per core) xt[128, 256 + 8*1024] uint8: a 256B weight block
(fp16 M tiles, fp8 M_h/M_r tiles), then 8 sample blocks of 1024B per
partition; each 256B-wide chunk mixes fp16 rows (low partitions) with
pairs of fp8 rows (high partitions) so 224 fp16 + 576 fp8 rows pack with
zero partition waste and every matmul operand starts at partition 0/32/64:
  [A0: 128 fp16 | mix64: p<64 fp16, p>=64 2xfp8 | mix32: p<32 fp16,
   p>=32 2xfp8 | Bfull: 2xfp8]

Input loads ride SWDGE identity gathers (prepare_only + trigger_dma),
which skips both HWDGE descriptor generation and the DGE->DMA handoff:
the stream starts ~1us earlier and each slice's completion semaphore
gates PE directly (Tile does not thread reader deps through prepared
gathers, so the first matmul of each slice waits explicitly). The gather
index tile idx[p, s] = 16s + (p%16) is built on device (two iotas + DVE
mask/add, bounced through a Pool copy so the descriptor generation is
ordered after it) and must be replicated across all eight 16-partition
groups -- each Q7 core reads its own group's copy. APs are int32-cast:
the cost model charges prepare time per element, and integer views skip
the non-finite checker (int64 breaks the 16-bit-granularity ucode).

Output path: PSUM -> SBUF fp16 copies on DVE into a [128, 1, 128] tile
(sample p's 8x16 feature blocks), stored by a single dma_scatter_add
prepared up front and fired by trigger_dma right after the last copy.
No DRAM pre-zero is needed: run_bass_kernel_spmd (native and
bass2jax/PJRT) hands the NEFF zero-filled output buffers, so the
scatter-add lands on zeros.
"""

import numpy as np
import ml_dtypes

import concourse.bass as bass
import concourse.tile as tile
from concourse import bacc, mybir
from concourse.bass_utils import run_bass_kernel_spmd

F8NP = ml_dtypes.float8_e4m3fn

N_CORES = 8
B = 8192
B_SHARD = B // N_CORES          # 1024
PIX = 784
NA = 224                        # fp16 rows (by descending energy)
NB8 = 576                       # fp8 rows incl 16 zero pads
NBLK = 8                        # sample blocks of 128 per core
BLKB = 1024                     # bytes per partition per block
NOUT = 10

# weight block layout (byte offsets within the first WB columns)
MA0_O = 0                       # [128,10] f16 rows perm[0:128]
MA64_O = 20                     # [64,10] f16 rows perm[128:192] (p<64)
MA32_O = 40                     # [32,10] f16 rows perm[192:224] (p<32)
MBF_O = 80                      # 4 x [128,10] f8: (full-lo, full-hi) x (h,r)
MB64_O = 120                    # 4 x [64,10] f8 on p64-127: (lo,hi) x (h,r)
MB32A_O = 120                   # 4 x [32,10] f8 on p32-63: (lo,hi) x (h,r)
MB32B_O = 160                   # 4 x [64,10] f8 on p64-127: (lo,hi) x (h,r)
WB = 256
NBYTES = WB + NBLK * BLKB

# per-block chunk byte offsets (within a block, per partition)
A0_O = 0                        # 256B f16: pixel perm[p]
M64_O = 256                     # 256B: p<64 f16 perm[128+p]; p>=64 two f8
M32_O = 512                     # 256B: p<32 f16 perm[192+p]; p>=32 two f8
BF_O = 768                      # 256B: two f8 rows (B[p], B[128+p])

# fp8 row index map (B[i] = perm[224+i], zeros for i >= 560):
#   full-lo B[0:128], full-hi B[128:256]
#   mix64-lo B[256:320] (p 64..127), mix64-hi B[320:384]
#   mix32-lo B[384:480] (p 32..127), mix32-hi B[480:576]

# input slices in blocks: small first (fast PE start), small last (short tail)
SLICES = ((0, 1), (1, 3), (3, 6), (6, 7), (7, 8))
PLAIN_INPUT = False


def _lc_dense(w, H, W_, oh, ow):
    """Dense [H*W_, oh*ow] matrix of one 3x3 locally-connected layer."""
    w = np.asarray(w, np.float64).reshape(oh, ow, 9)
    M = np.zeros((H * W_, oh * ow), np.float64)
    ox, oy = np.meshgrid(np.arange(oh), np.arange(ow), indexing="ij")
    col = (ox * ow + oy).ravel()
    for i in range(3):
        for j in range(3):
            row = ((ox + i) * W_ + (oy + j)).ravel()
            M[row, col] += w[:, :, i * 3 + j].ravel()
    return M


def _fold(w1, b1, w2, b2, fc_w, fc_b):
    W1 = _lc_dense(w1, 28, 28, 26, 26)          # [784, 676]
    W2 = _lc_dense(w2, 26, 26, 24, 24)          # [676, 576]
    fcw = np.asarray(fc_w, np.float64)          # [10, 576]
    M = W1 @ W2 @ fcw.T                         # [784, 10]
    c = (
        np.asarray(b1, np.float64).reshape(-1) @ W2
        + np.asarray(b2, np.float64).reshape(-1)
    ) @ fcw.T + np.asarray(fc_b, np.float64)    # [10]
    return M, c


def _build_bass():
    nc = bacc.Bacc("TRN2", target_bir_lowering=False, debug=False)
    u8 = mybir.dt.uint8
    f16 = mybir.dt.float16
    f8 = mybir.dt.float8e4
    f32 = mybir.dt.float32
    i16 = mybir.dt.int16
    i32 = mybir.dt.int32
    xt = nc.declare_dram_parameter("xt", [128, NBYTES], u8, isOutput=False)
    out = nc.declare_dram_parameter("out", [128, 8 * 16], f16, isOutput=True)

    with tile.TileContext(nc) as tc:
        with (
            tc.tile_pool(name="xp", bufs=len(SLICES)) as xp,
            tc.tile_pool(name="pp", bufs=NBLK // 2, space="PSUM") as pp,
            tc.tile_pool(name="op", bufs=3) as op,
        ):
            # Identity gather/scatter indices: idx[p, s] = 16s + (p % 16) --
            # row i at idxs[i%16, i//16], replicated across all eight
            # 16-partition groups (each Q7 core reads its own group's copy).
            # All idx ops stay on gpsimd: the prepared gathers' descriptor
            # generation only waits on the Pool engine semaphore (the
            # prepare-only dep demotion drops cross-engine idx deps).
            idxa = op.tile([128, 8], i16)
            nc.gpsimd.iota(idxa[:, :], [[16, 8]], base=0,
                           channel_multiplier=0)
            idxp = op.tile([128, 8], i16)
            nc.gpsimd.iota(idxp[:, :], [[0, 8]], base=0,
                           channel_multiplier=1)
            idxm = op.tile([128, 8], i16)
            nc.vector.tensor_scalar(idxm[:, :], idxp[:, :], 15, None,
                                    mybir.AluOpType.bitwise_and)
            idxd = op.tile([128, 8], i16)
            nc.vector.tensor_add(idxd[:, :], idxa[:, :], idxm[:, :])
            # Bounce through a Pool copy: the prepared gathers' descriptor
            # generation only waits on the Pool engine semaphore, and this
            # copy's tick transitively orders it after the DVE idx math.
            idxt = op.tile([128, 8], i16)
            nc.gpsimd.tensor_copy(idxt[:, :], idxd[:, :])

            # Output staging tile, zeroed on the otherwise-idle Act engine
            # so DVE stays clear for the idx math. No ones row: a pad row
            # of the fp8 tier streams constant 1.0 with c as its weight row.
            o3 = op.tile([128, 1, 128], f16)
            nc.scalar.memzero(o3[:])

            # Input loads ride SWDGE identity gathers: prep + trigger skips
            # both the HWDGE descriptor-gen and the DGE->DMA handoff delay,
            # so the stream starts ~1us earlier than a dma_start could.
            xs = []
            gsems = []
            psums = []
            for si, (b0, b1) in enumerate(SLICES):
                w = (WB if si == 0 else 0) + (b1 - b0) * BLKB
                t = xp.tile([128, w], u8)
                if PLAIN_INPUT:
                    nc.sync.dma_start(
                        t[:, :],
                        xt[:, WB + b0 * BLKB - (WB if si == 0 else 0)
                           : WB + b1 * BLKB],
                    )
                    gsems.append(None)
                    xs.append(t)
                    continue
                gsem = nc.alloc_semaphore(f"gin{si}")
                gsems.append(gsem)
                # int32 APs: the prep's cost model charges per element, and
                # integer views skip the non-finite data check.
                nc.gpsimd.dma_gather(
                    t[:, :].bitcast(i32).unsqueeze(1),
                    xt[:, WB + b0 * BLKB - (WB if si == 0 else 0)
                       : WB + b1 * BLKB].bitcast(i32),
                    idxt[:, :],
                    128,
                    128,
                    w // 4,
                    elem_step=NBYTES // 4,
                    prepare_only=True,
                    sem=gsem,
                )
                nc.gpsimd.trigger_dma(count=None)
                xs.append(t)
            t0 = xs[0]

            # Prepare the output scatter descriptors up front; only the
            # trigger (after the last copy) sits on the tail.
            dma_sem = nc.alloc_semaphore("oscat")
            nc.gpsimd.dma_scatter_add(
                out[:, :],
                o3[:, :, :],
                idxt[:, :],
                128,
                128,
                128,
                prepare_only=True,
                sem=dma_sem,
            )
            # No DRAM pre-zero needed: run_bass_kernel_spmd (native and
            # bass2jax/PJRT) hands the NEFF zero-filled output buffers, so
            # the scatter-add lands on zeros.

            def mf16(off, p0, p1):
                return t0[p0:p1, off : off + 2 * NOUT].bitcast(f16)

            def mf8(off, p0, p1):
                return t0[p0:p1, off : off + NOUT].bitcast(f8)

            ma0 = mf16(MA0_O, 0, 128)
            ma64 = mf16(MA64_O, 0, 64)
            ma32 = mf16(MA32_O, 0, 32)
            mbf = [mf8(MBF_O + i * NOUT, 0, 128) for i in range(4)]
            mb64 = [mf8(MB64_O + i * NOUT, 64, 128) for i in range(4)]
            mb32a = [mf8(MB32A_O + i * NOUT, 32, 64) for i in range(4)]
            mb32b = [mf8(MB32B_O + i * NOUT, 64, 128) for i in range(4)]

            # Tile does not thread reader deps through prepared gathers, so
            # gate PE explicitly on each gather's completion semaphore. The
            # warm matmul then absorbs the slice-0 wait once so every real
            # matmul waits on one semaphore lane.
            if not PLAIN_INPUT:
                nc.tensor.wait_ge(gsems[0], 16)
            wm = pp.tile([NOUT, 2], f32, bufs=1)
            nc.tensor.matmul(wm[:, 0:1], ma0, ma0[:, 0:1], start=True, stop=True)

            for si, (bb0, bb1) in enumerate(SLICES):
                base = WB if si == 0 else 0
                xsl = xs[si]
                if si > 0 and not PLAIN_INPUT:
                    nc.tensor.wait_ge(gsems[si], 16)
                for b in range(bb0, bb1):
                    cb = base + (b - bb0) * BLKB
                    if b % 2 == 0:
                        ps2 = pp.tile([128, 2, NOUT], f32)
                        psums.append(ps2)
                    ps = psums[b // 2][:, b % 2, :]

                    def x16(off, p0, p1):
                        return xsl[p0:p1, cb + off : cb + off + 256].bitcast(f16)

                    def x8(off, p0, p1):
                        return xsl[p0:p1, cb + off : cb + off + 128].bitcast(f8)

                    mm = nc.tensor.matmul
                    mm(ps[:], x16(A0_O, 0, 128), ma0, start=True, stop=False)
                    mm(ps[:], x16(M64_O, 0, 64), ma64, start=False, stop=False)
                    mm(ps[:], x16(M32_O, 0, 32), ma32, start=False, stop=False)
                    # fp8 tier: h and r passes share each x tile
                    for ci, off in enumerate((BF_O, BF_O + 128)):
                        xa = x8(off, 0, 128)
                        mm(ps[:], xa, mbf[2 * ci], start=False, stop=False)
                        mm(ps[:], xa, mbf[2 * ci + 1], start=False, stop=False)
                    for ci, off in enumerate((M64_O, M64_O + 128)):
                        xa = x8(off, 64, 128)
                        mm(ps[:], xa, mb64[2 * ci], start=False, stop=False)
                        mm(ps[:], xa, mb64[2 * ci + 1], start=False, stop=False)
                    for ci, off in enumerate((M32_O, M32_O + 128)):
                        xa = x8(off, 32, 64)
                        mm(ps[:], xa, mb32a[2 * ci], start=False, stop=False)
                        mm(ps[:], xa, mb32a[2 * ci + 1], start=False, stop=False)
                        xb = x8(off, 64, 128)
                        mm(ps[:], xb, mb32b[2 * ci], start=False, stop=False)
                        mm(ps[:], xb, mb32b[2 * ci + 1], start=False,
                           stop=(ci == 1))
                    # PSUM -> SBUF fp16 downcast into the scatter source,
                    # one copy per PSUM bank (= two blocks) to amortize the
                    # PSUM access bubble.
                    if b % 2 == 1:
                        dst = o3[:, 0:1, (b - 1) * 16 : (b + 1) * 16].rearrange(
                            "p a (b2 f) -> p (a b2) f", b2=2
                        )[:, :, 0:NOUT]
                        nc.vector.tensor_scalar_add(
                            dst, psums[b // 2][:, :, :], 0.0
                        )
            # Fire the prepared output scatter right behind the last copy.
            nc.gpsimd.trigger_dma(count=None)
    nc.finalize()
    return nc


def _prepare(inputs):
    x = np.asarray(inputs["x"], np.float32).reshape(B, PIX)
    M, c = _fold(
        inputs["w1"], inputs["b1"], inputs["w2"], inputs["b2"],
        inputs["fc_w"], inputs["fc_b"],
    )
    perm = np.argsort(-(M**2).sum(axis=1), kind="stable")
    Mp = M[perm]
    assert np.abs(Mp).max() < 200.0

    x16 = x[:, perm[:NA]].astype(np.float16)                  # [B, 224]
    x8 = np.zeros((B, NB8), F8NP)
    x8[:, : PIX - NA] = x[:, perm[NA:]].astype(F8NP)          # [B, 576]
    x8[:, PIX - NA] = 1.0              # bias row: constant 1.0
    M8 = np.zeros((NB8, NOUT), np.float64)
    M8[: PIX - NA] = Mp[NA:]
    M8[PIX - NA] = c                   # bias weights ride the first pad row
    M8h = M8.astype(np.float32).astype(F8NP)
    M8r = (M8 - M8h.astype(np.float64)).astype(np.float32).astype(F8NP)

    def u8v(a):
        return np.ascontiguousarray(a).view(np.uint8)

    wb = np.zeros((128, WB), np.uint8)
    wb[:, MA0_O : MA0_O + 20] = u8v(Mp[:128].astype(np.float16))
    wb[:64, MA64_O : MA64_O + 20] = u8v(Mp[128:192].astype(np.float16))
    wb[:32, MA32_O : MA32_O + 20] = u8v(Mp[192:224].astype(np.float16))
    for i, r0 in enumerate((0, 128)):       # full-lo, full-hi
        wb[:, MBF_O + 2 * i * NOUT : MBF_O + (2 * i + 1) * NOUT] = (
            u8v(M8h[r0 : r0 + 128])
        )
        wb[:, MBF_O + (2 * i + 1) * NOUT : MBF_O + (2 * i + 2) * NOUT] = (
            u8v(M8r[r0 : r0 + 128])
        )
    for i, r0 in enumerate((256, 320)):     # mix64 lo, hi (p 64..127)
        wb[64:, MB64_O + 2 * i * NOUT : MB64_O + (2 * i + 1) * NOUT] = (
            u8v(M8h[r0 : r0 + 64])
        )
        wb[64:, MB64_O + (2 * i + 1) * NOUT : MB64_O + (2 * i + 2) * NOUT] = (
            u8v(M8r[r0 : r0 + 64])
        )
    for i, r0 in enumerate((384, 480)):     # mix32 lo, hi: p 32..63 part
        wb[32:64, MB32A_O + 2 * i * NOUT : MB32A_O + (2 * i + 1) * NOUT] = (
            u8v(M8h[r0 : r0 + 32])
        )
        wb[32:64, MB32A_O + (2 * i + 1) * NOUT : MB32A_O + (2 * i + 2) * NOUT] = (
            u8v(M8r[r0 : r0 + 32])
        )
    for i, r0 in enumerate((416, 512)):     # mix32 lo, hi: p 64..127 part
        wb[64:, MB32B_O + 2 * i * NOUT : MB32B_O + (2 * i + 1) * NOUT] = (
            u8v(M8h[r0 : r0 + 64])
        )
        wb[64:, MB32B_O + (2 * i + 1) * NOUT : MB32B_O + (2 * i + 2) * NOUT] = (
            u8v(M8r[r0 : r0 + 64])
        )

    in_maps = []
    for ci in range(N_CORES):
        arr = np.empty((128, NBYTES), np.uint8)
        arr[:, :WB] = wb
        for b in range(NBLK):
            s0 = ci * B_SHARD + b * 128
            cb = WB + b * BLKB
            xa = x16[s0 : s0 + 128]                          # [128s, 224]
            xb = x8[s0 : s0 + 128]                           # [128s, 576]
            arr[:, cb : cb + 256] = u8v(xa[:, :128].T.copy()).reshape(128, 256)
            arr[:64, cb + M64_O : cb + M64_O + 256] = (
                u8v(xa[:, 128:192].T.copy()).reshape(64, 256)
            )
            arr[64:, cb + M64_O : cb + M64_O + 128] = u8v(xb[:, 256:320].T.copy())
            arr[64:, cb + M64_O + 128 : cb + M64_O + 256] = u8v(xb[:, 320:384].T.copy())
            arr[:32, cb + M32_O : cb + M32_O + 256] = (
                u8v(xa[:, 192:224].T.copy()).reshape(32, 256)
            )
            arr[32:, cb + M32_O : cb + M32_O + 128] = u8v(xb[:, 384:480].T.copy())
            arr[32:, cb + M32_O + 128 : cb + M32_O + 256] = u8v(xb[:, 480:576].T.copy())
            arr[:, cb + BF_O : cb + BF_O + 128] = u8v(xb[:, 0:128].T.copy())
            arr[:, cb + BF_O + 128 : cb + BF_O + 256] = u8v(xb[:, 128:256].T.copy())
        in_maps.append({"xt": arr})
    return in_maps


def _unpack(res):
    outs = []
    for i in range(N_CORES):
        o = np.asarray(res.results[i]["out"]).reshape(128, 8, 16)[:, :, :NOUT]
        outs.append(o.transpose(1, 0, 2).reshape(B_SHARD, NOUT))
    return np.concatenate(outs, axis=0).astype(np.float32)


def _build_for_sim(inputs):
    return _build_bass(), _prepare(inputs)[0]


def _run(inputs, trace=False, trace_cores=None):
    in_maps = _prepare(inputs)
    nc = _build_bass()
    res = run_bass_kernel_spmd(
        nc,
        in_maps,
        list(range(N_CORES)),
        trace=trace,
        trace_cores=trace_cores,
    )
    return _unpack(res), res


def kernel(**inputs) -> np.ndarray:
    out, _ = _run(inputs, trace=False)
    return out
